# revision 48
# baseline (speedup 1.0000x reference)
"""Trainium2 Bass kernel for PixelPropagationModule (per-pixel self-attention).

Math per batch sample b (B=8, C=256, CI=64, N=H*W=3136):
    Q = Wq @ x + bq            [CI, N]
    K = Wk @ x + bk            [CI, N]
    V = Wv @ x                 [C,  N]   (gamma folded into Wv; bv deferred)
    score[i, j] = sum_o Q[o, i] K[o, j]          (N x N)
    att = softmax(score, axis=j)
    out = gamma * (V @ att^T) + (x + gamma * bv)  -> [C, N]

Sharding: pure data parallel, one sample per NeuronCore (B == 8 == n_cores).

Device dataflow (per core) — 64x128 PE-array tiling everywhere in the
attention phase:
  - The score matmul has contraction CI=64, so the 128x128 PE array is
    reconfigured as two independent 64x128 tiles (T0: SBUF partitions 0-63,
    T8: partitions 64-127).  Each j-chunk PAIR (even chunk staged on
    partitions 0-63, odd chunk on partitions 64-127) computes two score
    matmuls CONCURRENTLY -> ~2x score throughput.  Q is duplicated onto both
    partition halves (free: the projection weight matrix is column-duplicated
    so the Q/K projections emit the duplicated/split layouts directly).
  - The out-matmul (contraction j=128 per chunk) is split into two
    64-contraction halves on the same T0/T8 tiles, accumulating into two
    separate PSUM accumulators po_lo/po_hi which are summed once per i-group
    during the (already required) PSUM->SBUF normalize pass.  This keeps the
    whole attention phase in one tile mode (no PE drains).
  - softmax runs without max subtraction (|score| <= ~40: exp safe in f32);
    the denominator s_i comes from an all-ones [64,128] stationary matmul
    over the DVE-accumulated exp sums, which lands s_i PRE-BROADCAST over
    all 128 psum partitions, so 1/s is a plain elementwise pass.
    Normalization is applied to the [C, N] output (deferred, flash-style).
  - gamma is folded into Wv on the host; gamma*bv is folded into the
    residual input xf = x + gamma*bv (exact: V's bias contributes
    bv * sum_j att_norm = bv post-normalization).

PSUM budget (8 banks of 2KiB/partition):
    pspair pool: 2 x [128, 1024] f32  (score pair: T0 at 0:448, T8 at
                 512:960; also rotates for the Q/K projection chunks and the
                 s-reduce)                                   = 4 banks
    po pool:     2 x [128, 1024] f32  (po_lo, po_hi: c-halves at 0/512)
                                                             = 4 banks
"""

import numpy as np
import ml_dtypes

import bass_rust as _bass_rust

import concourse.bass as bass
import concourse.mybir as mybir
import concourse.tile as tile
from concourse.bass_utils import run_bass_kernel_spmd

BF16 = mybir.dt.bfloat16
F32 = mybir.dt.float32
FP8 = mybir.dt.float8e4
NP_BF16 = ml_dtypes.bfloat16
NP_FP8 = ml_dtypes.float8_e4m3   # TRN FP8_EXP4: max +-240, has inf
AF = mybir.ActivationFunctionType

B, C, H, W = 8, 256, 56, 56
CI = 64
N = H * W            # 3136
NCORES = 8
PFD = 512            # projection chunk: 6 * 512 + 64 = 3136
# i-groups: 6 x 512 + 1 x 64.  512-wide psum regions fill banks exactly, and
# the tiny last group makes the kernel tail (exp -> s-chain -> normalize ->
# DMA drain) ~7x shorter than a uniform-width split would.
GROUPS = [(g * 512, 512) for g in range(6)] + [(3072, 64)]
FDMAX = 512
NJ = 25              # j-chunks: 24 x 128 + 1 x 64
NPAIR = 13           # 12 full pairs + 1 single (chunk 24, 64 wide)
OFF2 = 512           # second-half element offset inside [128, 1024] psum


def build_kernel(n_repeat: int = 1, hw_loop: bool = False,
                 sim_shrink: bool = False) -> bass.Bass:
    # sim_shrink: cost-model aid only -- TimelineSim charges concurrent
    # 64x128-tile matmuls serially, so shrink the T8-side matmuls to 16-wide
    # (deps preserved, ~zero sim cost) to approximate real concurrency.
    SW = 16 if sim_shrink else None
    nc = bass.Bass()

    xb_d = nc.declare_dram_parameter("xb", [C, N], FP8, isOutput=False)
    xf_d = nc.declare_dram_parameter("xf", [C, N], F32, isOutput=False)
    wq_d = nc.declare_dram_parameter("wq2T", [C, 128], FP8, isOutput=False)
    wk_d = nc.declare_dram_parameter("wk2T", [C, 128], FP8, isOutput=False)
    wv_d = nc.declare_dram_parameter("wvT", [C, C], FP8, isOutput=False)
    bq_d = nc.declare_dram_parameter("bq2", [128, 1], F32, isOutput=False)
    bk_d = nc.declare_dram_parameter("bk2", [128, 1], F32, isOutput=False)
    out_d = nc.declare_dram_parameter("out", [C, N], F32, isOutput=True)

    xb_r = xb_d[:].rearrange("(o p) n -> p o n", p=128)    # [128, 2, N] bf16
    xf_r = xf_d[:].rearrange("(o p) n -> p o n", p=128)    # [128, 2, N] f32
    out_r = out_d[:].rearrange("(o p) n -> p o n", p=128)  # [128, 2, N] f32

    with tile.TileContext(nc) as tc:
        with (
            tc.tile_pool(name="const", bufs=1) as cpool,
            tc.tile_pool(name="data", bufs=1) as dpool,
            tc.tile_pool(name="att", bufs=6) as apool,
            tc.tile_pool(name="accp", bufs=2) as accpool,
            tc.tile_pool(name="outp", bufs=3) as opool,
            tc.tile_pool(name="misc", bufs=3) as mpool,
            tc.tile_pool(name="ps_a", bufs=2, space="PSUM") as ps_a,
            tc.tile_pool(name="ps_o", bufs=2, space="PSUM") as ps_o,
        ):
            # warm tile memset first so the HAM warm-up matmuls can issue
            # while the input DMAs are still in flight
            warm_sb = cpool.tile([128, 512], BF16, name="warm_sb")
            nc.vector.memset(warm_sb[:], 0.0)

            # ---- constants / weights ----
            wq_sb = cpool.tile([128, 2, 128], FP8, name="wq_sb")
            nc.sync.dma_start(wq_sb[:], wq_d[:].rearrange("(o p) m -> p o m", p=128))
            wk_sb = cpool.tile([128, 2, 128], FP8, name="wk_sb")
            nc.sync.dma_start(wk_sb[:], wk_d[:].rearrange("(o p) m -> p o m", p=128))
            wv_sb = cpool.tile([128, 2, C], FP8, name="wv_sb")
            nc.sync.dma_start(wv_sb[:], wv_d[:].rearrange("(o p) m -> p o m", p=128))
            bq_sb = cpool.tile([128, 1], F32, name="bq_sb")
            nc.sync.dma_start(bq_sb[:], bq_d[:])
            bk_sb = cpool.tile([128, 1], F32, name="bk_sb")
            nc.sync.dma_start(bk_sb[:], bk_d[:])
            ones2 = cpool.tile([128, 128], BF16, name="ones2")
            nc.vector.memset(ones2[:], 1.0)

            # ---- x in SBUF (chunked so projections start early) ----
            xb_sb = dpool.tile([128, 2, N], FP8, name="xb_sb")
            xb_edges = [0, 128, 256, 512] + [512 * t for t in range(2, 7)] + [N]
            for e0, e1 in zip(xb_edges[:-1], xb_edges[1:]):
                nc.sync.dma_start(xb_sb[:, :, e0:e1], xb_r[:, :, e0:e1])
            xf_sb = dpool.tile([128, 2, N], F32, name="xf_sb")

            # warm the PE HAM clock gate during the initial x DMA wait:
            # dummy matmuls on a zeroed scratch tile (results never read)
            pwarm = ps_a.tile([128, 1024], F32, tag="ps_a")
            for wi in range(26):
                nc.tensor.matmul(pwarm[:, 0:256], lhsT=warm_sb[:, 0:128],
                                 rhs=warm_sb[:, 0:256], start=True, stop=True)

            # residual input: needed only from the first group's tail on,
            # so emit after xb so it does not steal early DMA bandwidth
            nc.sync.dma_start(xf_sb[:], xf_r)

            # q duplicated on both partition halves; k pairs split even/odd
            q2_sb = dpool.tile([128, N], BF16, name="q2_sb")
            k2_sb = dpool.tile([128, 13 * 128], BF16, name="k2_sb")
            # pair 12 has no odd chunk and its T0 weight slice is read 128
            # wide (cols 1600:1664 never written) -> zero once
            nc.vector.memset(k2_sb[:, 1600:1664], 0.0)
            vt_sb = dpool.tile([128, NJ, C], BF16, name="vt_sb")

            def _emit_body():
                # ---- projections, interleaved by x-DMA arrival ----
                vt_done = 0
                for t in range(7):
                    w = PFD if t < 6 else 64
                    sl = slice(t * PFD, t * PFD + w)
                    pq = ps_a.tile([128, 1024], F32, tag="ps_a")
                    nc.tensor.matmul(pq[:, 0:w], lhsT=wq_sb[:],
                                     rhs=xb_sb[:, :, sl], start=True, stop=True,
                                     perf_mode=mybir.MatmulPerfMode.DoubleRow)
                    nc.tensor.matmul(pq[:, OFF2:OFF2 + w], lhsT=wk_sb[:],
                                     rhs=xb_sb[:, :, sl], start=True, stop=True,
                                     perf_mode=mybir.MatmulPerfMode.DoubleRow)
                    nc.scalar.activation(q2_sb[:, sl], pq[:, 0:w],
                                         AF.Identity, bias=bq_sb[:])
                    if t < 6:
                        # K chunk covers j-chunks 4t..4t+3 = pairs 2t, 2t+1.
                        # even chunks (blocks 0,2) -> partitions 0-63;
                        # odd chunks (blocks 1,3) -> partitions 64-127.
                        pk = pq[:, OFF2:OFF2 + PFD].rearrange(
                            "p (c two x) -> p two c x", two=2, x=128)
                        ksl = slice(t * 256, t * 256 + 256)
                        kd = k2_sb[:, ksl].rearrange("p (c x) -> p c x", x=128)
                        nc.scalar.activation(kd[0:64], pk[0:64, 0],
                                             AF.Identity, bias=bk_sb[0:64])
                        nc.scalar.activation(kd[64:128], pk[64:128, 1],
                                             AF.Identity, bias=bk_sb[64:128])
                    else:
                        # tail: j-chunk 24 (even, pair 12, T0 only)
                        nc.scalar.activation(k2_sb[0:64, 1536:1600],
                                             pq[0:64, OFF2:OFF2 + 64],
                                             AF.Identity, bias=bk_sb[0:64])
                    # V^T tiles: vt_sb[p, jt, c] = gamma*V[c, jt*128+p]
                    vt_avail = min(NJ, ((t + 1) * PFD) // 128) if t < 6 else NJ
                    for jt in range(vt_done, vt_avail):
                        jsz = 128 if jt < NJ - 1 else 64
                        j0 = jt * 128
                        pv = ps_o.tile([128, 1024], F32, tag="ps_o")
                        pvt = pv[:jsz, 0:C]
                        nc.tensor.matmul(pvt, lhsT=xb_sb[:, 0, j0:j0 + jsz],
                                         rhs=wv_sb[:, 0, :], start=True, stop=False)
                        nc.tensor.matmul(pvt, lhsT=xb_sb[:, 1, j0:j0 + jsz],
                                         rhs=wv_sb[:, 1, :], start=False, stop=True)
                        nc.vector.tensor_copy(vt_sb[:jsz, jt, :], pvt)
                    vt_done = vt_avail

                # ---- attention: i-groups, 64x128-tiled ----
                defer = []
                for g, (i0, fd) in enumerate(GROUPS):
                    isl = slice(i0, i0 + fd)
                    last_g = g == len(GROUPS) - 1
                    po_lo = ps_o.tile([128, 1024], F32, tag="ps_o", name="po_lo")
                    po_hi = ps_o.tile([128, 1024], F32, tag="ps_o", name="po_hi")
                    acc = accpool.tile([128, 2, FDMAX], BF16, tag="acc")
                    att_tiles = {}

                    def emit_out_mms(t, pars=(0, 1), att_tiles=att_tiles,
                                     po_lo=po_lo, po_hi=po_hi, fd=fd):
                        # out-mms for pair t: j-chunks 2t (even) and 2t+1
                        # (odd), each split into T0 (j 0-63) and T8 (j 64-127)
                        # halves accumulating into po_lo / po_hi.  State is
                        # bound via default args: carried calls run in the
                        # NEXT group's iteration scope.
                        att = att_tiles[t]
                        if pars[-1] == 1 or 2 * t + 1 >= NJ:
                            att_tiles.pop(t)
                        for par, jt in [(p, 2 * t + p) for p in pars]:
                            if jt >= NJ:
                                continue
                            stop_lo = jt == 24
                            stop_hi = jt == 23
                            for cc in range(2):
                                osl = slice(cc * OFF2, cc * OFF2 + fd)
                                nc.tensor.matmul(
                                    po_lo[:, osl],
                                    lhsT=vt_sb[0:64, jt, cc * 128:(cc + 1) * 128],
                                    rhs=att[0:64, par, 0:fd],
                                    start=(t == 0 and par == 0),
                                    stop=stop_lo)
                                if jt < 24:
                                    w8 = min(SW or fd, fd)
                                    nc.tensor.matmul(
                                        po_hi[:, cc * OFF2:cc * OFF2 + w8],
                                        lhsT=vt_sb[64:128, jt, cc * 128:(cc + 1) * 128],
                                        rhs=att[64:128, par, 0:w8],
                                        start=(t == 0 and par == 0),
                                        stop=stop_hi)

                    pending = []
                    ps12 = None
                    for t in range(NPAIR):
                        lastp = t == NPAIR - 1
                        ps = ps_a.tile([128, 1024], F32, tag="ps_a")
                        if lastp:
                            ps12 = ps
                        att = apool.tile([128, 2, FDMAX], BF16, tag="att")
                        nc.tensor.matmul(ps[:, 0:fd],
                                         lhsT=k2_sb[0:64, t * 128:(t + 1) * 128],
                                         rhs=q2_sb[0:64, isl],
                                         start=True, stop=True)
                        if not lastp:
                            w8 = min(SW or fd, fd)
                            nc.tensor.matmul(ps[:, OFF2:OFF2 + w8],
                                             lhsT=k2_sb[64:128, t * 128:(t + 1) * 128],
                                             rhs=q2_sb[64:128, i0:i0 + w8],
                                             start=True, stop=True)
                        if t == 1 and defer:
                            # previous group's s-reduce + drain chain runs
                            # in this group's ACT-paced ramp
                            defer.pop(0)()
                        if len(pending) >= 2:
                            emit_out_mms(pending.pop(0))
                        psv = ps[:].rearrange("p (h x) -> p h x", h=2)[:, :, 0:fd]
                        if not lastp:
                            nc.scalar.activation(att[:, :, 0:fd], psv, AF.Exp)
                            if t == 0:
                                nc.vector.tensor_copy(acc[:, :, 0:fd],
                                                      att[:, :, 0:fd])
                            else:
                                nc.vector.tensor_add(acc[:, :, 0:fd],
                                                     acc[:, :, 0:fd],
                                                     att[:, :, 0:fd])
                        else:
                            # chunk 24: only T0 half is meaningful; folded
                            # into the s-reduce matmul directly
                            nc.scalar.activation(att[0:64, 0, 0:fd],
                                                 psv[0:64, 0], AF.Exp)
                            att_last = att
                        att_tiles[t] = att
                        pending.append(t)

                    po_lov = po_lo[:].rearrange("p (h x) -> p h x", h=2)[:, :, 0:fd]
                    po_hiv = po_hi[:].rearrange("p (h x) -> p h x", h=2)[:, :, 0:fd]
                    # acc halves pre-folded on DVE: shortens the s-reduce
                    # matmul streams
                    acc2 = mpool.tile([128, FDMAX], BF16, tag="acc2")
                    nc.vector.tensor_add(acc2[:, 0:fd], acc[:, 0, 0:fd],
                                         acc[:, 1, 0:fd])
                    # pair 11's out-mms fill the PE while exp(12) / the
                    # last acc-add are still in flight on ACT/DVE
                    emit_out_mms(pending.pop(0))
                    # drain po_hi via DVE (ACT paces the group interiors)
                    hi_sb = opool.tile([128, 2, FDMAX], F32, tag="hi",
                                       name="hi_sb")
                    nc.vector.tensor_copy(hi_sb[:, :, 0:fd], po_hiv)
                    for p in pending:
                        emit_out_mms(p)
                    pending = []

                    def endgame(fd=fd, i0=i0, po_lo=po_lo, po_hi=po_hi,
                                po_lov=po_lov, po_hiv=po_hiv, hi_sb=hi_sb,
                                acc2=acc2, att_last=att_last):
                        # s-reduce: all-ones [64,128] stationary matmuls emit
                        # the denominators pre-broadcast over all partitions.
                        # Targets live in po_hi's banks (drained by hi-copy
                        # just above), NOT in a score-pair psum slot, so the
                        # next group's pairs never wait on the s-chain.
                        sA = po_hi[:, OFF2:OFF2 + fd]
                        nc.tensor.matmul(sA, lhsT=ones2[0:64, :],
                                         rhs=acc2[0:64, 0:fd],
                                         start=True, stop=False)
                        nc.tensor.matmul(sA, lhsT=ones2[0:64, :],
                                         rhs=att_last[0:64, 0, 0:fd],
                                         start=False, stop=True)
                        w8 = min(SW or fd, fd)
                        nc.tensor.matmul(po_hi[:, 0:w8], lhsT=ones2[64:128, :],
                                         rhs=acc2[64:128, 0:w8],
                                         start=True, stop=True)
                        s2_sb = mpool.tile([128, 2, FDMAX], F32, tag="s2_sb")
                        nc.vector.tensor_copy(
                            s2_sb[:, :, 0:fd], po_hi[:].rearrange(
                                "p (h x) -> p h x", h=2)[:, :, 0:fd])
                        ob_sb = opool.tile([128, 2, FDMAX], BF16, tag="ob",
                                           name="ob_sb")
                        out_sb = opool.tile([128, 2, FDMAX], F32, tag="out")
                        with nc.allow_low_precision(
                                reason="attention output is gamma-damped; "
                                       "bf16 merge is well within tolerance"):
                            nc.vector.tensor_add(ob_sb[:, :, 0:fd], po_lov,
                                                 hi_sb[:, :, 0:fd])
                        s_sb = mpool.tile([128, FDMAX], F32, tag="s_sb")
                        nc.vector.tensor_add(s_sb[:, 0:fd], s2_sb[:, 0, 0:fd],
                                             s2_sb[:, 1, 0:fd])
                        inv_sb = mpool.tile([128, FDMAX], BF16, tag="inv")
                        with nc.allow_low_precision(
                                reason="1/s feeds the gamma-damped attention "
                                       "path; bf16 is well within tolerance"):
                            nc.vector.reciprocal(inv_sb[:, 0:fd], s_sb[:, 0:fd])
                        nc.gpsimd.tensor_mul(
                            ob_sb[:, :, 0:fd], ob_sb[:, :, 0:fd],
                            inv_sb[:, None, 0:fd].to_broadcast((128, 2, fd)))
                        nc.gpsimd.tensor_add(out_sb[:, :, 0:fd],
                                             ob_sb[:, :, 0:fd],
                                             xf_sb[:, :, i0:i0 + fd])
                        nc.sync.dma_start(out_r[:, :, i0:i0 + fd],
                                          out_sb[:, :, 0:fd])

                    if g >= len(GROUPS) - 2:
                        # the 64-wide last group is too small to absorb a
                        # deferred chain; run the last two endgames inline
                        endgame()
                    else:
                        defer.append(endgame)

            if hw_loop:
                with tc.For_i(0, n_repeat):
                    _emit_body()
            else:
                for _rep in range(n_repeat):
                    _emit_body()

    # TRN2 allows at most one semaphore wait per instruction; Tile can emit
    # more. Split them (EventSemaphore chains) like Bacc.compile() does.
    _bass_rust.move_matmul_waits_to_ldweights(nc.m)
    _bass_rust.generate_event_semaphores(nc)
    return nc


_CACHED = {}


def _get_kernel(n_repeat: int = 1) -> bass.Bass:
    if n_repeat not in _CACHED:
        _CACHED[n_repeat] = build_kernel(n_repeat)
    return _CACHED[n_repeat]


def make_in_maps(x, Wq, bq, Wk, bk, Wv, bv, gamma):
    x = np.asarray(x, dtype=np.float32)
    Wq = np.asarray(Wq, dtype=np.float32)
    bq = np.asarray(bq, dtype=np.float32)
    Wk = np.asarray(Wk, dtype=np.float32)
    bk = np.asarray(bk, dtype=np.float32)
    Wv = np.asarray(Wv, dtype=np.float32)
    bv = np.asarray(bv, dtype=np.float32)
    g = float(np.asarray(gamma, dtype=np.float32).reshape(-1)[0])

    def q8(a):
        return np.clip(a, -240, 240).astype(NP_FP8)

    wq2T = np.ascontiguousarray(
        q8(np.concatenate([Wq.T, Wq.T], axis=1)))               # [C, 128]
    wk2T = np.ascontiguousarray(
        q8(np.concatenate([Wk.T, Wk.T], axis=1)))               # [C, 128]
    wvT = np.ascontiguousarray(q8((g * Wv).T))                  # [C, C]
    bq2 = np.ascontiguousarray(
        np.concatenate([bq, bq]).reshape(128, 1))               # [128, 1] f32
    bk2 = np.ascontiguousarray(
        np.concatenate([bk, bk]).reshape(128, 1))

    xf = np.ascontiguousarray(
        x.reshape(B, C, N) + (g * bv)[None, :, None])           # x + gamma*bv
    xbf = np.ascontiguousarray(q8(x.reshape(B, C, N)))

    in_maps = []
    for b in range(B):
        in_maps.append({
            "xb": xbf[b],
            "xf": xf[b],
            "wq2T": wq2T,
            "wk2T": wk2T,
            "wvT": wvT,
            "bq2": bq2,
            "bk2": bk2,
        })
    return in_maps


def kernel(x, Wq, bq, Wk, bk, Wv, bv, gamma):
    in_maps = make_in_maps(x, Wq, bq, Wk, bk, Wv, bv, gamma)
    nc = _get_kernel(1)
    res = run_bass_kernel_spmd(nc, in_maps, core_ids=list(range(NCORES)))
    out = np.stack([res.results[b]["out"] for b in range(B)], axis=0)
    return out.reshape(B, C, H, W).astype(np.float32)


# revision 56
# speedup vs baseline: 1.0110x; 1.0110x over previous
"""Trainium2 Bass kernel for PixelPropagationModule (per-pixel self-attention).

Math per batch sample b (B=8, C=256, CI=64, N=H*W=3136):
    Q = Wq @ x + bq            [CI, N]
    K = Wk @ x + bk            [CI, N]
    V = Wv @ x                 [C,  N]   (gamma folded into Wv; bv deferred)
    score[i, j] = sum_o Q[o, i] K[o, j]          (N x N)
    att = softmax(score, axis=j)
    out = gamma * (V @ att^T) + (x + gamma * bv)  -> [C, N]

Sharding: pure data parallel, one sample per NeuronCore (B == 8 == n_cores).

Device dataflow (per core) — 64x128 PE-array tiling everywhere in the
attention phase:
  - The score matmul has contraction CI=64, so the 128x128 PE array is
    reconfigured as two independent 64x128 tiles (T0: SBUF partitions 0-63,
    T8: partitions 64-127).  Each j-chunk PAIR (even chunk staged on
    partitions 0-63, odd chunk on partitions 64-127) computes two score
    matmuls CONCURRENTLY -> ~2x score throughput.  Q is duplicated onto both
    partition halves (free: the projection weight matrix is column-duplicated
    so the Q/K projections emit the duplicated/split layouts directly).
  - The out-matmul (contraction j=128 per chunk) is split into two
    64-contraction halves on the same T0/T8 tiles, accumulating into two
    separate PSUM accumulators po_lo/po_hi which are summed once per i-group
    during the (already required) PSUM->SBUF normalize pass.  This keeps the
    whole attention phase in one tile mode (no PE drains).
  - softmax runs without max subtraction (|score| <= ~40: exp safe in f32);
    the denominator s_i comes from an all-ones [64,128] stationary matmul
    over the DVE-accumulated exp sums, which lands s_i PRE-BROADCAST over
    all 128 psum partitions, so 1/s is a plain elementwise pass.
    Normalization is applied to the [C, N] output (deferred, flash-style).
  - gamma is folded into Wv on the host; gamma*bv is folded into the
    residual input xf = x + gamma*bv (exact: V's bias contributes
    bv * sum_j att_norm = bv post-normalization).

PSUM budget (8 banks of 2KiB/partition):
    pspair pool: 2 x [128, 1024] f32  (score pair: T0 at 0:448, T8 at
                 512:960; also rotates for the Q/K projection chunks and the
                 s-reduce)                                   = 4 banks
    po pool:     2 x [128, 1024] f32  (po_lo, po_hi: c-halves at 0/512)
                                                             = 4 banks
"""

import numpy as np
import ml_dtypes

import bass_rust as _bass_rust

import concourse.bass as bass
import concourse.mybir as mybir
import concourse.tile as tile
from concourse.bass_utils import run_bass_kernel_spmd

BF16 = mybir.dt.bfloat16
F32 = mybir.dt.float32
FP8 = mybir.dt.float8e4
NP_BF16 = ml_dtypes.bfloat16
NP_FP8 = ml_dtypes.float8_e4m3   # TRN FP8_EXP4: max +-240, has inf
AF = mybir.ActivationFunctionType

B, C, H, W = 8, 256, 56, 56
CI = 64
N = H * W            # 3136
NCORES = 8
PFD = 512            # projection chunk: 6 * 512 + 64 = 3136
# i-groups: 5 x 512 + 416 + 160.  512-wide psum regions fill banks exactly;
# tapering the last two groups balances the kernel-tail drain chain against
# the per-instruction ACT overhead of narrow groups (swept in sim).
GROUPS = [(g * 512, 512) for g in range(5)] + [(2560, 416), (2976, 160)]
FDMAX = 512
NJ = 25              # j-chunks: 24 x 128 + 1 x 64
NPAIR = 13           # 12 full pairs + 1 single (chunk 24, 64 wide)
OFF2 = 512           # second-half element offset inside [128, 1024] psum


def build_kernel(n_repeat: int = 1, hw_loop: bool = False,
                 sim_shrink: bool = False) -> bass.Bass:
    # sim_shrink: cost-model aid only -- TimelineSim charges concurrent
    # 64x128-tile matmuls serially, so shrink the T8-side matmuls to 16-wide
    # (deps preserved, ~zero sim cost) to approximate real concurrency.
    SW = 16 if sim_shrink else None
    nc = bass.Bass()

    xb_d = nc.declare_dram_parameter("xb", [C, N], FP8, isOutput=False)
    xf_d = nc.declare_dram_parameter("xf", [C, N], F32, isOutput=False)
    wq_d = nc.declare_dram_parameter("wq2T", [C, 128], FP8, isOutput=False)
    wk_d = nc.declare_dram_parameter("wk2T", [C, 128], FP8, isOutput=False)
    wv_d = nc.declare_dram_parameter("wvT", [C, C], FP8, isOutput=False)
    bq_d = nc.declare_dram_parameter("bq2", [128, 1], F32, isOutput=False)
    bk_d = nc.declare_dram_parameter("bk2", [128, 1], F32, isOutput=False)
    out_d = nc.declare_dram_parameter("out", [C, N], F32, isOutput=True)

    xb_r = xb_d[:].rearrange("(o p) n -> p o n", p=128)    # [128, 2, N] bf16
    xf_r = xf_d[:].rearrange("(o p) n -> p o n", p=128)    # [128, 2, N] f32
    out_r = out_d[:].rearrange("(o p) n -> p o n", p=128)  # [128, 2, N] f32

    with tile.TileContext(nc) as tc:
        with (
            tc.tile_pool(name="const", bufs=1) as cpool,
            tc.tile_pool(name="data", bufs=1) as dpool,
            tc.tile_pool(name="att", bufs=6) as apool,
            tc.tile_pool(name="accp", bufs=2) as accpool,
            tc.tile_pool(name="outp", bufs=3) as opool,
            tc.tile_pool(name="misc", bufs=3) as mpool,
            tc.tile_pool(name="ps_a", bufs=2, space="PSUM") as ps_a,
            tc.tile_pool(name="ps_o", bufs=2, space="PSUM") as ps_o,
        ):
            # warm tile memset first so the HAM warm-up matmuls can issue
            # while the input DMAs are still in flight
            warm_sb = cpool.tile([128, 512], BF16, name="warm_sb")
            nc.vector.memset(warm_sb[:], 0.0)

            # ---- constants / weights ----
            wq_sb = cpool.tile([128, 2, 128], FP8, name="wq_sb")
            nc.sync.dma_start(wq_sb[:], wq_d[:].rearrange("(o p) m -> p o m", p=128))
            wk_sb = cpool.tile([128, 2, 128], FP8, name="wk_sb")
            nc.sync.dma_start(wk_sb[:], wk_d[:].rearrange("(o p) m -> p o m", p=128))
            wv_sb = cpool.tile([128, 2, C], FP8, name="wv_sb")
            nc.sync.dma_start(wv_sb[:], wv_d[:].rearrange("(o p) m -> p o m", p=128))
            bq_sb = cpool.tile([128, 1], F32, name="bq_sb")
            nc.sync.dma_start(bq_sb[:], bq_d[:])
            bk_sb = cpool.tile([128, 1], F32, name="bk_sb")
            nc.sync.dma_start(bk_sb[:], bk_d[:])
            ones2 = cpool.tile([128, 128], BF16, name="ones2")
            nc.vector.memset(ones2[:], 1.0)

            # ---- x in SBUF (chunked per projection-consumer width; finer
            # head edges add HWDGE overhead without earlier starts since the
            # first Q/K matmul reads the full [0:512] slice) ----
            xb_sb = dpool.tile([128, 2, N], FP8, name="xb_sb")
            xb_edges = [0, 512] + [512 * t for t in range(2, 7)] + [N]
            for e0, e1 in zip(xb_edges[:-1], xb_edges[1:]):
                nc.sync.dma_start(xb_sb[:, :, e0:e1], xb_r[:, :, e0:e1])
            xf_sb = dpool.tile([128, 2, N], F32, name="xf_sb")

            # warm the PE HAM clock gate during the initial x DMA wait:
            # dummy matmuls on a zeroed scratch tile (results never read)
            pwarm = ps_a.tile([128, 1024], F32, tag="ps_a")
            for wi in range(26):
                nc.tensor.matmul(pwarm[:, 0:256], lhsT=warm_sb[:, 0:128],
                                 rhs=warm_sb[:, 0:256], start=True, stop=True)

            # residual input: needed only from the first group's tail on,
            # so emit after xb so it does not steal early DMA bandwidth
            nc.sync.dma_start(xf_sb[:], xf_r)

            # q duplicated on both partition halves; k pairs split even/odd
            q2_sb = dpool.tile([128, N], BF16, name="q2_sb")
            k2_sb = dpool.tile([128, 13 * 128], BF16, name="k2_sb")
            # pair 12 has no odd chunk and its T0 weight slice is read 128
            # wide (cols 1600:1664 never written) -> zero once
            nc.vector.memset(k2_sb[:, 1600:1664], 0.0)
            vt_sb = dpool.tile([128, NJ, C], BF16, name="vt_sb")

            def _emit_body():
                # ---- projections, interleaved by x-DMA arrival ----
                vt_done = 0
                for t in range(7):
                    w = PFD if t < 6 else 64
                    sl = slice(t * PFD, t * PFD + w)
                    pq = ps_a.tile([128, 1024], F32, tag="ps_a")
                    nc.tensor.matmul(pq[:, 0:w], lhsT=wq_sb[:],
                                     rhs=xb_sb[:, :, sl], start=True, stop=True,
                                     perf_mode=mybir.MatmulPerfMode.DoubleRow)
                    nc.tensor.matmul(pq[:, OFF2:OFF2 + w], lhsT=wk_sb[:],
                                     rhs=xb_sb[:, :, sl], start=True, stop=True,
                                     perf_mode=mybir.MatmulPerfMode.DoubleRow)
                    nc.scalar.activation(q2_sb[:, sl], pq[:, 0:w],
                                         AF.Identity, bias=bq_sb[:])
                    if t < 6:
                        # K chunk covers j-chunks 4t..4t+3 = pairs 2t, 2t+1.
                        # even chunks (blocks 0,2) -> partitions 0-63;
                        # odd chunks (blocks 1,3) -> partitions 64-127.
                        pk = pq[:, OFF2:OFF2 + PFD].rearrange(
                            "p (c two x) -> p two c x", two=2, x=128)
                        ksl = slice(t * 256, t * 256 + 256)
                        kd = k2_sb[:, ksl].rearrange("p (c x) -> p c x", x=128)
                        nc.scalar.activation(kd[0:64], pk[0:64, 0],
                                             AF.Identity, bias=bk_sb[0:64])
                        nc.scalar.activation(kd[64:128], pk[64:128, 1],
                                             AF.Identity, bias=bk_sb[64:128])
                    else:
                        # tail: j-chunk 24 (even, pair 12, T0 only)
                        nc.scalar.activation(k2_sb[0:64, 1536:1600],
                                             pq[0:64, OFF2:OFF2 + 64],
                                             AF.Identity, bias=bk_sb[0:64])
                    # V^T tiles: vt_sb[p, jt, c] = gamma*V[c, jt*128+p]
                    vt_avail = min(NJ, ((t + 1) * PFD) // 128) if t < 6 else NJ
                    for jt in range(vt_done, vt_avail):
                        jsz = 128 if jt < NJ - 1 else 64
                        j0 = jt * 128
                        pv = ps_o.tile([128, 1024], F32, tag="ps_o")
                        pvt = pv[:jsz, 0:C]
                        nc.tensor.matmul(pvt, lhsT=xb_sb[:, 0, j0:j0 + jsz],
                                         rhs=wv_sb[:, 0, :], start=True, stop=False)
                        nc.tensor.matmul(pvt, lhsT=xb_sb[:, 1, j0:j0 + jsz],
                                         rhs=wv_sb[:, 1, :], start=False, stop=True)
                        nc.vector.tensor_copy(vt_sb[:jsz, jt, :], pvt)
                    vt_done = vt_avail

                # ---- attention: i-groups, 64x128-tiled ----
                defer = []
                for g, (i0, fd) in enumerate(GROUPS):
                    isl = slice(i0, i0 + fd)
                    last_g = g == len(GROUPS) - 1
                    po_lo = ps_o.tile([128, 1024], F32, tag="ps_o", name="po_lo")
                    po_hi = ps_o.tile([128, 1024], F32, tag="ps_o", name="po_hi")
                    acc = accpool.tile([128, 2, FDMAX], BF16, tag="acc")
                    att_tiles = {}

                    def emit_out_mms(t, pars=(0, 1), att_tiles=att_tiles,
                                     po_lo=po_lo, po_hi=po_hi, fd=fd):
                        # out-mms for pair t: j-chunks 2t (even) and 2t+1
                        # (odd), each split into T0 (j 0-63) and T8 (j 64-127)
                        # halves accumulating into po_lo / po_hi.  State is
                        # bound via default args: carried calls run in the
                        # NEXT group's iteration scope.
                        att = att_tiles[t]
                        if pars[-1] == 1 or 2 * t + 1 >= NJ:
                            att_tiles.pop(t)
                        for par, jt in [(p, 2 * t + p) for p in pars]:
                            if jt >= NJ:
                                continue
                            stop_lo = jt == 24
                            stop_hi = jt == 23
                            for cc in range(2):
                                osl = slice(cc * OFF2, cc * OFF2 + fd)
                                nc.tensor.matmul(
                                    po_lo[:, osl],
                                    lhsT=vt_sb[0:64, jt, cc * 128:(cc + 1) * 128],
                                    rhs=att[0:64, par, 0:fd],
                                    start=(t == 0 and par == 0),
                                    stop=stop_lo)
                                if jt < 24:
                                    w8 = min(SW or fd, fd)
                                    nc.tensor.matmul(
                                        po_hi[:, cc * OFF2:cc * OFF2 + w8],
                                        lhsT=vt_sb[64:128, jt, cc * 128:(cc + 1) * 128],
                                        rhs=att[64:128, par, 0:w8],
                                        start=(t == 0 and par == 0),
                                        stop=stop_hi)

                    pending = []
                    ps12 = None
                    for t in range(NPAIR):
                        lastp = t == NPAIR - 1
                        ps = ps_a.tile([128, 1024], F32, tag="ps_a")
                        if lastp:
                            ps12 = ps
                        att = apool.tile([128, 2, FDMAX], BF16, tag="att")
                        nc.tensor.matmul(ps[:, 0:fd],
                                         lhsT=k2_sb[0:64, t * 128:(t + 1) * 128],
                                         rhs=q2_sb[0:64, isl],
                                         start=True, stop=True)
                        if not lastp:
                            w8 = min(SW or fd, fd)
                            nc.tensor.matmul(ps[:, OFF2:OFF2 + w8],
                                             lhsT=k2_sb[64:128, t * 128:(t + 1) * 128],
                                             rhs=q2_sb[64:128, i0:i0 + w8],
                                             start=True, stop=True)
                        if t == 1 and defer:
                            # previous group's s-reduce + drain chain runs
                            # in this group's ACT-paced ramp
                            defer.pop(0)()
                        if len(pending) >= 2:
                            emit_out_mms(pending.pop(0))
                        psv = ps[:].rearrange("p (h x) -> p h x", h=2)[:, :, 0:fd]
                        if not lastp:
                            nc.scalar.activation(att[:, :, 0:fd], psv, AF.Exp)
                            if t == 0:
                                nc.vector.tensor_copy(acc[:, :, 0:fd],
                                                      att[:, :, 0:fd])
                            else:
                                nc.vector.tensor_add(acc[:, :, 0:fd],
                                                     acc[:, :, 0:fd],
                                                     att[:, :, 0:fd])
                        else:
                            # chunk 24: only T0 half is meaningful; folded
                            # into the s-reduce matmul directly
                            nc.scalar.activation(att[0:64, 0, 0:fd],
                                                 psv[0:64, 0], AF.Exp)
                            att_last = att
                        att_tiles[t] = att
                        pending.append(t)

                    po_lov = po_lo[:].rearrange("p (h x) -> p h x", h=2)[:, :, 0:fd]
                    po_hiv = po_hi[:].rearrange("p (h x) -> p h x", h=2)[:, :, 0:fd]
                    # acc halves pre-folded on DVE: shortens the s-reduce
                    # matmul streams
                    acc2 = mpool.tile([128, FDMAX], BF16, tag="acc2")
                    nc.vector.tensor_add(acc2[:, 0:fd], acc[:, 0, 0:fd],
                                         acc[:, 1, 0:fd])
                    # pair 11's out-mms fill the PE while exp(12) / the
                    # last acc-add are still in flight on ACT/DVE
                    emit_out_mms(pending.pop(0))
                    # drain po_hi via DVE (ACT paces the group interiors)
                    hi_sb = opool.tile([128, 2, FDMAX], F32, tag="hi",
                                       name="hi_sb")
                    nc.vector.tensor_copy(hi_sb[:, :, 0:fd], po_hiv)
                    for p in pending:
                        emit_out_mms(p)
                    pending = []

                    def endgame(fd=fd, i0=i0, po_lo=po_lo, po_hi=po_hi,
                                po_lov=po_lov, po_hiv=po_hiv, hi_sb=hi_sb,
                                acc2=acc2, att_last=att_last):
                        # s-reduce: all-ones [64,128] stationary matmuls emit
                        # the denominators pre-broadcast over all partitions.
                        # Targets live in po_hi's banks (drained by hi-copy
                        # just above), NOT in a score-pair psum slot, so the
                        # next group's pairs never wait on the s-chain.
                        sA = po_hi[:, OFF2:OFF2 + fd]
                        nc.tensor.matmul(sA, lhsT=ones2[0:64, :],
                                         rhs=acc2[0:64, 0:fd],
                                         start=True, stop=False)
                        nc.tensor.matmul(sA, lhsT=ones2[0:64, :],
                                         rhs=att_last[0:64, 0, 0:fd],
                                         start=False, stop=True)
                        w8 = min(SW or fd, fd)
                        nc.tensor.matmul(po_hi[:, 0:w8], lhsT=ones2[64:128, :],
                                         rhs=acc2[64:128, 0:w8],
                                         start=True, stop=True)
                        s2_sb = mpool.tile([128, 2, FDMAX], F32, tag="s2_sb")
                        nc.vector.tensor_copy(
                            s2_sb[:, :, 0:fd], po_hi[:].rearrange(
                                "p (h x) -> p h x", h=2)[:, :, 0:fd])
                        ob_sb = opool.tile([128, 2, FDMAX], BF16, tag="ob",
                                           name="ob_sb")
                        out_sb = opool.tile([128, 2, FDMAX], F32, tag="out")
                        with nc.allow_low_precision(
                                reason="attention output is gamma-damped; "
                                       "bf16 merge is well within tolerance"):
                            nc.vector.tensor_add(ob_sb[:, :, 0:fd], po_lov,
                                                 hi_sb[:, :, 0:fd])
                        s_sb = mpool.tile([128, FDMAX], F32, tag="s_sb")
                        nc.vector.tensor_add(s_sb[:, 0:fd], s2_sb[:, 0, 0:fd],
                                             s2_sb[:, 1, 0:fd])
                        inv_sb = mpool.tile([128, FDMAX], BF16, tag="inv")
                        with nc.allow_low_precision(
                                reason="1/s feeds the gamma-damped attention "
                                       "path; bf16 is well within tolerance"):
                            nc.vector.reciprocal(inv_sb[:, 0:fd], s_sb[:, 0:fd])
                        nc.gpsimd.tensor_mul(
                            ob_sb[:, :, 0:fd], ob_sb[:, :, 0:fd],
                            inv_sb[:, None, 0:fd].to_broadcast((128, 2, fd)))
                        nc.gpsimd.tensor_add(out_sb[:, :, 0:fd],
                                             ob_sb[:, :, 0:fd],
                                             xf_sb[:, :, i0:i0 + fd])
                        nc.sync.dma_start(out_r[:, :, i0:i0 + fd],
                                          out_sb[:, :, 0:fd])

                    if g >= len(GROUPS) - 2:
                        # the 64-wide last group is too small to absorb a
                        # deferred chain; run the last two endgames inline
                        endgame()
                    else:
                        defer.append(endgame)

            if hw_loop:
                with tc.For_i(0, n_repeat):
                    _emit_body()
            else:
                for _rep in range(n_repeat):
                    _emit_body()

    # TRN2 allows at most one semaphore wait per instruction; Tile can emit
    # more. Split them (EventSemaphore chains) like Bacc.compile() does.
    _bass_rust.move_matmul_waits_to_ldweights(nc.m)
    _bass_rust.generate_event_semaphores(nc)
    return nc


_CACHED = {}


def _get_kernel(n_repeat: int = 1) -> bass.Bass:
    if n_repeat not in _CACHED:
        _CACHED[n_repeat] = build_kernel(n_repeat)
    return _CACHED[n_repeat]


def make_in_maps(x, Wq, bq, Wk, bk, Wv, bv, gamma):
    x = np.asarray(x, dtype=np.float32)
    Wq = np.asarray(Wq, dtype=np.float32)
    bq = np.asarray(bq, dtype=np.float32)
    Wk = np.asarray(Wk, dtype=np.float32)
    bk = np.asarray(bk, dtype=np.float32)
    Wv = np.asarray(Wv, dtype=np.float32)
    bv = np.asarray(bv, dtype=np.float32)
    g = float(np.asarray(gamma, dtype=np.float32).reshape(-1)[0])

    def q8(a):
        return np.clip(a, -240, 240).astype(NP_FP8)

    wq2T = np.ascontiguousarray(
        q8(np.concatenate([Wq.T, Wq.T], axis=1)))               # [C, 128]
    wk2T = np.ascontiguousarray(
        q8(np.concatenate([Wk.T, Wk.T], axis=1)))               # [C, 128]
    wvT = np.ascontiguousarray(q8((g * Wv).T))                  # [C, C]
    bq2 = np.ascontiguousarray(
        np.concatenate([bq, bq]).reshape(128, 1))               # [128, 1] f32
    bk2 = np.ascontiguousarray(
        np.concatenate([bk, bk]).reshape(128, 1))

    xf = np.ascontiguousarray(
        x.reshape(B, C, N) + (g * bv)[None, :, None])           # x + gamma*bv
    xbf = np.ascontiguousarray(q8(x.reshape(B, C, N)))

    in_maps = []
    for b in range(B):
        in_maps.append({
            "xb": xbf[b],
            "xf": xf[b],
            "wq2T": wq2T,
            "wk2T": wk2T,
            "wvT": wvT,
            "bq2": bq2,
            "bk2": bk2,
        })
    return in_maps


def kernel(x, Wq, bq, Wk, bk, Wv, bv, gamma):
    in_maps = make_in_maps(x, Wq, bq, Wk, bk, Wv, bv, gamma)
    nc = _get_kernel(1)
    res = run_bass_kernel_spmd(nc, in_maps, core_ids=list(range(NCORES)))
    out = np.stack([res.results[b]["out"] for b in range(B)], axis=0)
    return out.reshape(B, C, H, W).astype(np.float32)


# revision 57
# speedup vs baseline: 1.0118x; 1.0007x over previous
"""Trainium2 Bass kernel for PixelPropagationModule (per-pixel self-attention).

Math per batch sample b (B=8, C=256, CI=64, N=H*W=3136):
    Q = Wq @ x + bq            [CI, N]
    K = Wk @ x + bk            [CI, N]
    V = Wv @ x                 [C,  N]   (gamma folded into Wv; bv deferred)
    score[i, j] = sum_o Q[o, i] K[o, j]          (N x N)
    att = softmax(score, axis=j)
    out = gamma * (V @ att^T) + (x + gamma * bv)  -> [C, N]

Sharding: pure data parallel, one sample per NeuronCore (B == 8 == n_cores).

Device dataflow (per core) — 64x128 PE-array tiling everywhere in the
attention phase:
  - The score matmul has contraction CI=64, so the 128x128 PE array is
    reconfigured as two independent 64x128 tiles (T0: SBUF partitions 0-63,
    T8: partitions 64-127).  Each j-chunk PAIR (even chunk staged on
    partitions 0-63, odd chunk on partitions 64-127) computes two score
    matmuls CONCURRENTLY -> ~2x score throughput.  Q is duplicated onto both
    partition halves (free: the projection weight matrix is column-duplicated
    so the Q/K projections emit the duplicated/split layouts directly).
  - The out-matmul (contraction j=128 per chunk) is split into two
    64-contraction halves on the same T0/T8 tiles, accumulating into two
    separate PSUM accumulators po_lo/po_hi which are summed once per i-group
    during the (already required) PSUM->SBUF normalize pass.  This keeps the
    whole attention phase in one tile mode (no PE drains).
  - softmax runs without max subtraction (|score| <= ~40: exp safe in f32);
    the denominator s_i comes from an all-ones [64,128] stationary matmul
    over the DVE-accumulated exp sums, which lands s_i PRE-BROADCAST over
    all 128 psum partitions, so 1/s is a plain elementwise pass.
    Normalization is applied to the [C, N] output (deferred, flash-style).
  - gamma is folded into Wv on the host; gamma*bv is folded into the
    residual input xf = x + gamma*bv (exact: V's bias contributes
    bv * sum_j att_norm = bv post-normalization).

PSUM budget (8 banks of 2KiB/partition):
    pspair pool: 2 x [128, 1024] f32  (score pair: T0 at 0:448, T8 at
                 512:960; also rotates for the Q/K projection chunks and the
                 s-reduce)                                   = 4 banks
    po pool:     2 x [128, 1024] f32  (po_lo, po_hi: c-halves at 0/512)
                                                             = 4 banks
"""

import numpy as np
import ml_dtypes

import bass_rust as _bass_rust

import concourse.bass as bass
import concourse.mybir as mybir
import concourse.tile as tile
from concourse.bass_utils import run_bass_kernel_spmd

BF16 = mybir.dt.bfloat16
F32 = mybir.dt.float32
FP8 = mybir.dt.float8e4
NP_BF16 = ml_dtypes.bfloat16
NP_FP8 = ml_dtypes.float8_e4m3   # TRN FP8_EXP4: max +-240, has inf
AF = mybir.ActivationFunctionType

B, C, H, W = 8, 256, 56, 56
CI = 64
N = H * W            # 3136
NCORES = 8
PFD = 512            # projection chunk: 6 * 512 + 64 = 3136
# i-groups: 5 x 512 + 416 + 160.  512-wide psum regions fill banks exactly;
# tapering the last two groups balances the kernel-tail drain chain against
# the per-instruction ACT overhead of narrow groups (swept in sim).
GROUPS = [(g * 512, 512) for g in range(5)] + [(2560, 416), (2976, 160)]
FDMAX = 512
NJ = 25              # j-chunks: 24 x 128 + 1 x 64
NPAIR = 13           # 12 full pairs + 1 single (chunk 24, 64 wide)
OFF2 = 512           # second-half element offset inside [128, 1024] psum


def build_kernel(n_repeat: int = 1, hw_loop: bool = False,
                 sim_shrink: bool = False) -> bass.Bass:
    # sim_shrink: cost-model aid only -- TimelineSim charges concurrent
    # 64x128-tile matmuls serially, so shrink the T8-side matmuls to 16-wide
    # (deps preserved, ~zero sim cost) to approximate real concurrency.
    SW = 16 if sim_shrink else None
    nc = bass.Bass()

    xb_d = nc.declare_dram_parameter("xb", [C, N], FP8, isOutput=False)
    xf_d = nc.declare_dram_parameter("xf", [C, N], F32, isOutput=False)
    wq_d = nc.declare_dram_parameter("wq2T", [C, 128], FP8, isOutput=False)
    wk_d = nc.declare_dram_parameter("wk2T", [C, 128], FP8, isOutput=False)
    wv_d = nc.declare_dram_parameter("wvT", [C, C], FP8, isOutput=False)
    bq_d = nc.declare_dram_parameter("bq2", [128, 1], F32, isOutput=False)
    bk_d = nc.declare_dram_parameter("bk2", [128, 1], F32, isOutput=False)
    out_d = nc.declare_dram_parameter("out", [C, N], F32, isOutput=True)

    xb_r = xb_d[:].rearrange("(o p) n -> p o n", p=128)    # [128, 2, N] bf16
    xf_r = xf_d[:].rearrange("(o p) n -> p o n", p=128)    # [128, 2, N] f32
    out_r = out_d[:].rearrange("(o p) n -> p o n", p=128)  # [128, 2, N] f32

    with tile.TileContext(nc) as tc:
        with (
            tc.tile_pool(name="const", bufs=1) as cpool,
            tc.tile_pool(name="data", bufs=1) as dpool,
            tc.tile_pool(name="att", bufs=6) as apool,
            tc.tile_pool(name="accp", bufs=2) as accpool,
            tc.tile_pool(name="outp", bufs=3) as opool,
            tc.tile_pool(name="misc", bufs=3) as mpool,
            tc.tile_pool(name="ps_a", bufs=2, space="PSUM") as ps_a,
            tc.tile_pool(name="ps_o", bufs=2, space="PSUM") as ps_o,
        ):
            # warm tile memset first so the HAM warm-up matmuls can issue
            # while the input DMAs are still in flight
            warm_sb = cpool.tile([128, 512], BF16, name="warm_sb")
            nc.vector.memset(warm_sb[:], 0.0)

            # ---- constants / weights ----
            wq_sb = cpool.tile([128, 2, 128], FP8, name="wq_sb")
            nc.sync.dma_start(wq_sb[:], wq_d[:].rearrange("(o p) m -> p o m", p=128))
            wk_sb = cpool.tile([128, 2, 128], FP8, name="wk_sb")
            nc.sync.dma_start(wk_sb[:], wk_d[:].rearrange("(o p) m -> p o m", p=128))
            wv_sb = cpool.tile([128, 2, C], FP8, name="wv_sb")
            nc.sync.dma_start(wv_sb[:], wv_d[:].rearrange("(o p) m -> p o m", p=128))
            bq_sb = cpool.tile([128, 1], F32, name="bq_sb")
            nc.sync.dma_start(bq_sb[:], bq_d[:])
            bk_sb = cpool.tile([128, 1], F32, name="bk_sb")
            nc.sync.dma_start(bk_sb[:], bk_d[:])
            ones2 = cpool.tile([128, 128], BF16, name="ones2")
            nc.vector.memset(ones2[:], 1.0)

            # ---- x in SBUF (chunked per projection-consumer width; finer
            # head edges add HWDGE overhead without earlier starts since the
            # first Q/K matmul reads the full [0:512] slice) ----
            xb_sb = dpool.tile([128, 2, N], FP8, name="xb_sb")
            xb_edges = [0, 512] + [512 * t for t in range(2, 7)] + [N]
            for e0, e1 in zip(xb_edges[:-1], xb_edges[1:]):
                nc.sync.dma_start(xb_sb[:, :, e0:e1], xb_r[:, :, e0:e1])
            xf_sb = dpool.tile([128, 2, N], F32, name="xf_sb")

            # warm the PE HAM clock gate during the initial x DMA wait:
            # dummy matmuls on a zeroed scratch tile (results never read)
            pwarm = ps_a.tile([128, 1024], F32, tag="ps_a")
            for wi in range(20):
                nc.tensor.matmul(pwarm[:, 0:256], lhsT=warm_sb[:, 0:128],
                                 rhs=warm_sb[:, 0:256], start=True, stop=True)

            # residual input: needed only from the first group's tail on,
            # so emit after xb so it does not steal early DMA bandwidth
            nc.sync.dma_start(xf_sb[:], xf_r)

            # q duplicated on both partition halves; k pairs split even/odd
            q2_sb = dpool.tile([128, N], BF16, name="q2_sb")
            k2_sb = dpool.tile([128, 13 * 128], BF16, name="k2_sb")
            # pair 12 has no odd chunk and its T0 weight slice is read 128
            # wide (cols 1600:1664 never written) -> zero once
            nc.vector.memset(k2_sb[:, 1600:1664], 0.0)
            vt_sb = dpool.tile([128, NJ, C], BF16, name="vt_sb")

            def _emit_body():
                # ---- projections, interleaved by x-DMA arrival ----
                vt_done = 0
                for t in range(7):
                    w = PFD if t < 6 else 64
                    sl = slice(t * PFD, t * PFD + w)
                    pq = ps_a.tile([128, 1024], F32, tag="ps_a")
                    nc.tensor.matmul(pq[:, 0:w], lhsT=wq_sb[:],
                                     rhs=xb_sb[:, :, sl], start=True, stop=True,
                                     perf_mode=mybir.MatmulPerfMode.DoubleRow)
                    nc.tensor.matmul(pq[:, OFF2:OFF2 + w], lhsT=wk_sb[:],
                                     rhs=xb_sb[:, :, sl], start=True, stop=True,
                                     perf_mode=mybir.MatmulPerfMode.DoubleRow)
                    nc.scalar.activation(q2_sb[:, sl], pq[:, 0:w],
                                         AF.Identity, bias=bq_sb[:])
                    if t < 6:
                        # K chunk covers j-chunks 4t..4t+3 = pairs 2t, 2t+1.
                        # even chunks (blocks 0,2) -> partitions 0-63;
                        # odd chunks (blocks 1,3) -> partitions 64-127.
                        pk = pq[:, OFF2:OFF2 + PFD].rearrange(
                            "p (c two x) -> p two c x", two=2, x=128)
                        ksl = slice(t * 256, t * 256 + 256)
                        kd = k2_sb[:, ksl].rearrange("p (c x) -> p c x", x=128)
                        nc.scalar.activation(kd[0:64], pk[0:64, 0],
                                             AF.Identity, bias=bk_sb[0:64])
                        nc.scalar.activation(kd[64:128], pk[64:128, 1],
                                             AF.Identity, bias=bk_sb[64:128])
                    else:
                        # tail: j-chunk 24 (even, pair 12, T0 only)
                        nc.scalar.activation(k2_sb[0:64, 1536:1600],
                                             pq[0:64, OFF2:OFF2 + 64],
                                             AF.Identity, bias=bk_sb[0:64])
                    # V^T tiles: vt_sb[p, jt, c] = gamma*V[c, jt*128+p]
                    vt_avail = min(NJ, ((t + 1) * PFD) // 128) if t < 6 else NJ
                    for jt in range(vt_done, vt_avail):
                        jsz = 128 if jt < NJ - 1 else 64
                        j0 = jt * 128
                        pv = ps_o.tile([128, 1024], F32, tag="ps_o")
                        pvt = pv[:jsz, 0:C]
                        nc.tensor.matmul(pvt, lhsT=xb_sb[:, 0, j0:j0 + jsz],
                                         rhs=wv_sb[:, 0, :], start=True, stop=False)
                        nc.tensor.matmul(pvt, lhsT=xb_sb[:, 1, j0:j0 + jsz],
                                         rhs=wv_sb[:, 1, :], start=False, stop=True)
                        nc.vector.tensor_copy(vt_sb[:jsz, jt, :], pvt)
                    vt_done = vt_avail

                # ---- attention: i-groups, 64x128-tiled ----
                defer = []
                for g, (i0, fd) in enumerate(GROUPS):
                    isl = slice(i0, i0 + fd)
                    last_g = g == len(GROUPS) - 1
                    po_lo = ps_o.tile([128, 1024], F32, tag="ps_o", name="po_lo")
                    po_hi = ps_o.tile([128, 1024], F32, tag="ps_o", name="po_hi")
                    acc = accpool.tile([128, 2, FDMAX], BF16, tag="acc")
                    att_tiles = {}

                    def emit_out_mms(t, pars=(0, 1), att_tiles=att_tiles,
                                     po_lo=po_lo, po_hi=po_hi, fd=fd):
                        # out-mms for pair t: j-chunks 2t (even) and 2t+1
                        # (odd), each split into T0 (j 0-63) and T8 (j 64-127)
                        # halves accumulating into po_lo / po_hi.  State is
                        # bound via default args: carried calls run in the
                        # NEXT group's iteration scope.
                        att = att_tiles[t]
                        if pars[-1] == 1 or 2 * t + 1 >= NJ:
                            att_tiles.pop(t)
                        for par, jt in [(p, 2 * t + p) for p in pars]:
                            if jt >= NJ:
                                continue
                            stop_lo = jt == 24
                            stop_hi = jt == 23
                            for cc in range(2):
                                osl = slice(cc * OFF2, cc * OFF2 + fd)
                                nc.tensor.matmul(
                                    po_lo[:, osl],
                                    lhsT=vt_sb[0:64, jt, cc * 128:(cc + 1) * 128],
                                    rhs=att[0:64, par, 0:fd],
                                    start=(t == 0 and par == 0),
                                    stop=stop_lo)
                                if jt < 24:
                                    w8 = min(SW or fd, fd)
                                    nc.tensor.matmul(
                                        po_hi[:, cc * OFF2:cc * OFF2 + w8],
                                        lhsT=vt_sb[64:128, jt, cc * 128:(cc + 1) * 128],
                                        rhs=att[64:128, par, 0:w8],
                                        start=(t == 0 and par == 0),
                                        stop=stop_hi)

                    pending = []
                    ps12 = None
                    for t in range(NPAIR):
                        lastp = t == NPAIR - 1
                        ps = ps_a.tile([128, 1024], F32, tag="ps_a")
                        if lastp:
                            ps12 = ps
                        att = apool.tile([128, 2, FDMAX], BF16, tag="att")
                        nc.tensor.matmul(ps[:, 0:fd],
                                         lhsT=k2_sb[0:64, t * 128:(t + 1) * 128],
                                         rhs=q2_sb[0:64, isl],
                                         start=True, stop=True)
                        if not lastp:
                            w8 = min(SW or fd, fd)
                            nc.tensor.matmul(ps[:, OFF2:OFF2 + w8],
                                             lhsT=k2_sb[64:128, t * 128:(t + 1) * 128],
                                             rhs=q2_sb[64:128, i0:i0 + w8],
                                             start=True, stop=True)
                        if t == 1 and defer:
                            # previous group's s-reduce + drain chain runs
                            # in this group's ACT-paced ramp
                            defer.pop(0)()
                        if len(pending) >= 2:
                            emit_out_mms(pending.pop(0))
                        psv = ps[:].rearrange("p (h x) -> p h x", h=2)[:, :, 0:fd]
                        if not lastp:
                            nc.scalar.activation(att[:, :, 0:fd], psv, AF.Exp)
                            if t == 0:
                                nc.vector.tensor_copy(acc[:, :, 0:fd],
                                                      att[:, :, 0:fd])
                            else:
                                nc.vector.tensor_add(acc[:, :, 0:fd],
                                                     acc[:, :, 0:fd],
                                                     att[:, :, 0:fd])
                        else:
                            # chunk 24: only T0 half is meaningful; folded
                            # into the s-reduce matmul directly
                            nc.scalar.activation(att[0:64, 0, 0:fd],
                                                 psv[0:64, 0], AF.Exp)
                            att_last = att
                        att_tiles[t] = att
                        pending.append(t)

                    po_lov = po_lo[:].rearrange("p (h x) -> p h x", h=2)[:, :, 0:fd]
                    po_hiv = po_hi[:].rearrange("p (h x) -> p h x", h=2)[:, :, 0:fd]
                    # acc halves pre-folded on DVE: shortens the s-reduce
                    # matmul streams
                    acc2 = mpool.tile([128, FDMAX], BF16, tag="acc2")
                    nc.vector.tensor_add(acc2[:, 0:fd], acc[:, 0, 0:fd],
                                         acc[:, 1, 0:fd])
                    # pair 11's out-mms fill the PE while exp(12) / the
                    # last acc-add are still in flight on ACT/DVE
                    emit_out_mms(pending.pop(0))
                    # drain po_hi via DVE (ACT paces the group interiors)
                    hi_sb = opool.tile([128, 2, FDMAX], F32, tag="hi",
                                       name="hi_sb")
                    nc.vector.tensor_copy(hi_sb[:, :, 0:fd], po_hiv)
                    for p in pending:
                        emit_out_mms(p)
                    pending = []

                    def endgame(fd=fd, i0=i0, po_lo=po_lo, po_hi=po_hi,
                                po_lov=po_lov, po_hiv=po_hiv, hi_sb=hi_sb,
                                acc2=acc2, att_last=att_last):
                        # s-reduce: all-ones [64,128] stationary matmuls emit
                        # the denominators pre-broadcast over all partitions.
                        # Targets live in po_hi's banks (drained by hi-copy
                        # just above), NOT in a score-pair psum slot, so the
                        # next group's pairs never wait on the s-chain.
                        sA = po_hi[:, OFF2:OFF2 + fd]
                        nc.tensor.matmul(sA, lhsT=ones2[0:64, :],
                                         rhs=acc2[0:64, 0:fd],
                                         start=True, stop=False)
                        nc.tensor.matmul(sA, lhsT=ones2[0:64, :],
                                         rhs=att_last[0:64, 0, 0:fd],
                                         start=False, stop=True)
                        w8 = min(SW or fd, fd)
                        nc.tensor.matmul(po_hi[:, 0:w8], lhsT=ones2[64:128, :],
                                         rhs=acc2[64:128, 0:w8],
                                         start=True, stop=True)
                        s2_sb = mpool.tile([128, 2, FDMAX], F32, tag="s2_sb")
                        nc.vector.tensor_copy(
                            s2_sb[:, :, 0:fd], po_hi[:].rearrange(
                                "p (h x) -> p h x", h=2)[:, :, 0:fd])
                        ob_sb = opool.tile([128, 2, FDMAX], BF16, tag="ob",
                                           name="ob_sb")
                        out_sb = opool.tile([128, 2, FDMAX], F32, tag="out")
                        with nc.allow_low_precision(
                                reason="attention output is gamma-damped; "
                                       "bf16 merge is well within tolerance"):
                            nc.vector.tensor_add(ob_sb[:, :, 0:fd], po_lov,
                                                 hi_sb[:, :, 0:fd])
                        s_sb = mpool.tile([128, FDMAX], F32, tag="s_sb")
                        nc.vector.tensor_add(s_sb[:, 0:fd], s2_sb[:, 0, 0:fd],
                                             s2_sb[:, 1, 0:fd])
                        inv_sb = mpool.tile([128, FDMAX], BF16, tag="inv")
                        with nc.allow_low_precision(
                                reason="1/s feeds the gamma-damped attention "
                                       "path; bf16 is well within tolerance"):
                            nc.vector.reciprocal(inv_sb[:, 0:fd], s_sb[:, 0:fd])
                        nc.gpsimd.tensor_mul(
                            ob_sb[:, :, 0:fd], ob_sb[:, :, 0:fd],
                            inv_sb[:, None, 0:fd].to_broadcast((128, 2, fd)))
                        nc.gpsimd.tensor_add(out_sb[:, :, 0:fd],
                                             ob_sb[:, :, 0:fd],
                                             xf_sb[:, :, i0:i0 + fd])
                        nc.sync.dma_start(out_r[:, :, i0:i0 + fd],
                                          out_sb[:, :, 0:fd])

                    if g >= len(GROUPS) - 2:
                        # the 64-wide last group is too small to absorb a
                        # deferred chain; run the last two endgames inline
                        endgame()
                    else:
                        defer.append(endgame)

            if hw_loop:
                with tc.For_i(0, n_repeat):
                    _emit_body()
            else:
                for _rep in range(n_repeat):
                    _emit_body()

    # TRN2 allows at most one semaphore wait per instruction; Tile can emit
    # more. Split them (EventSemaphore chains) like Bacc.compile() does.
    _bass_rust.move_matmul_waits_to_ldweights(nc.m)
    _bass_rust.generate_event_semaphores(nc)
    return nc


_CACHED = {}


def _get_kernel(n_repeat: int = 1) -> bass.Bass:
    if n_repeat not in _CACHED:
        _CACHED[n_repeat] = build_kernel(n_repeat)
    return _CACHED[n_repeat]


def make_in_maps(x, Wq, bq, Wk, bk, Wv, bv, gamma):
    x = np.asarray(x, dtype=np.float32)
    Wq = np.asarray(Wq, dtype=np.float32)
    bq = np.asarray(bq, dtype=np.float32)
    Wk = np.asarray(Wk, dtype=np.float32)
    bk = np.asarray(bk, dtype=np.float32)
    Wv = np.asarray(Wv, dtype=np.float32)
    bv = np.asarray(bv, dtype=np.float32)
    g = float(np.asarray(gamma, dtype=np.float32).reshape(-1)[0])

    def q8(a):
        return np.clip(a, -240, 240).astype(NP_FP8)

    wq2T = np.ascontiguousarray(
        q8(np.concatenate([Wq.T, Wq.T], axis=1)))               # [C, 128]
    wk2T = np.ascontiguousarray(
        q8(np.concatenate([Wk.T, Wk.T], axis=1)))               # [C, 128]
    wvT = np.ascontiguousarray(q8((g * Wv).T))                  # [C, C]
    bq2 = np.ascontiguousarray(
        np.concatenate([bq, bq]).reshape(128, 1))               # [128, 1] f32
    bk2 = np.ascontiguousarray(
        np.concatenate([bk, bk]).reshape(128, 1))

    xf = np.ascontiguousarray(
        x.reshape(B, C, N) + (g * bv)[None, :, None])           # x + gamma*bv
    xbf = np.ascontiguousarray(q8(x.reshape(B, C, N)))

    in_maps = []
    for b in range(B):
        in_maps.append({
            "xb": xbf[b],
            "xf": xf[b],
            "wq2T": wq2T,
            "wk2T": wk2T,
            "wvT": wvT,
            "bq2": bq2,
            "bk2": bk2,
        })
    return in_maps


def kernel(x, Wq, bq, Wk, bk, Wv, bv, gamma):
    in_maps = make_in_maps(x, Wq, bq, Wk, bk, Wv, bv, gamma)
    nc = _get_kernel(1)
    res = run_bass_kernel_spmd(nc, in_maps, core_ids=list(range(NCORES)))
    out = np.stack([res.results[b]["out"] for b in range(B)], axis=0)
    return out.reshape(B, C, H, W).astype(np.float32)
